# revision 8
# baseline (speedup 1.0000x reference)
"""Trainium2 Bass kernel for nn_DIYloss_1709396984424 — v2.

Loss: for binary labels, mean over (one, zero) pairs of (1 + p[l] - p[k])^2
where p = sigmoid(pred_Y). The L^2 pairwise sum collapses to O(L) masked
reductions. With n1 = sum(m), s1 = sum(m*p), s2 = sum(m*p^2), S = sum(p),
A = sum(p^2 + 2p):

    num + den = n1*A + (L - 2*n1)*s2 + 2*s1^2 - 2*L*s1 - 2*s1*S + n1*(L - n1)
    loss      = (num + den) / den   where den = n1*(L - n1)

Schedule: five tiny PE partition-reductions fire the moment each stats
column lands, and the scalar epilogue is a running sum that picks up each
total in arrival order, so only ~50ns of scalar chain sits after the last
column total:
  SP  : input DMA first, final 4-byte reg store.
  ACT : act-table load (auto), sigmoid, Square-with-accum of (p+1) for
        the A' = A + L column, then eight ENGINE-0ns scalar ops gated by
        the early matmuls: d = L - n1, den = n1*d, u = L - 2*n1,
        cn1 = n1, cs1d = 2*s1, negcs1d = -2*s1, s1mL = s1 - L, and the
        late merge kA = n1*A' + k1 (ACT's sem send beats DVE's by 2ns
        and it unblocks k2 on DVE).
  DVE : const memsets, mask cast + n1 row sums (hidden in the DMA wait),
        mp = m*p (accum s1), pS = copy p (accum S; tensor_scalar runs in
        the 2x DVE mode at 93ns and fills the mp->mpp RAW gap),
        mpp = mp*p (accum s2), then the ENGINE-0ns epilogue
        nsq = n1^2 (den - L*n1 == -n1^2 absorbs the A' shift), r = 1/den,
        G0 = 2*s1*(s1-L) - nsq, k1 = -2*s1*S + G0, k2 = (L-2*n1)*s2,
        out = (k2 + kA)*r gated on a 2-count sem that provably forces
        both final merges.
  PE  : mm0 (n1, ~1.5us early), mm1 (s1), mmS (S), mm2 (s2), mmA (A',
        last) — each a [128,1]x[128,1] reduction into its own PSUM bank.
  Pool: nothing but the end-of-program DMA-queue + semaphore reset (its
        compute ops are not supported by this ISA).

ISA constraints learned the hard way: a DVE op may touch at most ONE PSUM
operand (two PSUM tensors fail BIR verification; PSUM scalars off
partition 0 too), and Pool/GPSIMD cannot run TensorScalarPtr at all. All
cross-total products therefore pair one PSUM total with SBUF cells that
ACT bounced out of earlier PSUM results (ACT scalar ops cost 0ns).

Hazard rules: engine pipelines do NOT interlock same-engine RAW on small
operands; every RAW dependency carries a semaphore edge (one wait per
instruction; counting sems + in-order retirement cover multi-producer
cases; cross-engine producers share a counting sem so one wait covers
both — a count of N implies all N increments happened, regardless of
arrival order).

The framework's startup barrier and three of its four const-tensor memsets
are suppressed at module-build time (the sigmoid bias uses the kept
const-0.0). The end-of-program reset rides on the final-value semaphore
rather than a store-completion sem so it overlaps the SP store.
"""

import numpy as np

try:
    import concourse.bass as cbass  # noqa: F401
except ImportError:  # pragma: no cover - grading env should have it on path
    import sys

    sys.path.insert(0, "/opt/trn_rl_repo")
    import concourse.bass as cbass  # noqa: F401

from concourse import bacc, mybir
from concourse.bass_utils import run_bass_kernel_spmd

L = 8192
P = 128
F = L // P  # 64
N_CORES = 8

_f32 = mybir.dt.float32
_i32 = mybir.dt.int32
_Alu = mybir.AluOpType
_Act = mybir.ActivationFunctionType
_X = mybir.AxisListType.X

_built = None

_suppress = {"on": False}
_orig_memset = cbass.BassSharedVectorInterface.memset
_orig_aeb = cbass.Bass.all_engine_barrier


def _memset_patched(self, ap, constant):
    if _suppress["on"] and constant != 0.0:
        return None
    return _orig_memset(self, ap, constant)


def _aeb_patched(self, *a, **k):
    if _suppress["on"]:
        return None
    return _orig_aeb(self, *a, **k)


cbass.BassSharedVectorInterface.memset = _memset_patched
cbass.Bass.all_engine_barrier = _aeb_patched


def _build():
    _suppress["on"] = True
    try:
        nc = bacc.Bacc(
            "TRN2", debug=False, target_bir_lowering=False, num_devices=N_CORES
        )
    finally:
        _suppress["on"] = False

    # cols 0:F = pred_Y (f32), cols F:2F = true_Y (int32 bitcast to f32)
    xin_d = nc.dram_tensor("xin", [P, 2 * F], _f32, kind="ExternalInput")
    out_d = nc.dram_tensor("out", [1, 1], _f32, kind="ExternalOutput")

    from contextlib import ExitStack

    with ExitStack() as stack:
        e = stack.enter_context
        xt = e(nc.sbuf_tensor("xt", [P, 2 * F], _f32))
        p = e(nc.sbuf_tensor("p", [P, F], _f32))
        m1 = e(nc.sbuf_tensor("m1", [P, F], _f32))
        mp = e(nc.sbuf_tensor("mp", [P, F], _f32))
        mpp = e(nc.sbuf_tensor("mpp", [P, F], _f32))
        p2A = e(nc.sbuf_tensor("p2A", [P, F], _f32))
        sc = e(nc.sbuf_tensor("sc", [P, F], _f32))  # Scopy dummy out
        stats = e(nc.sbuf_tensor("stats", [P, 4], _f32))
        ncol = e(nc.sbuf_tensor("ncol", [P, 1], _f32))
        ones = e(nc.sbuf_tensor("ones", [P, 1], _f32))
        cL = e(nc.sbuf_tensor("cL", [P, 1], _f32))     # +L
        cNL = e(nc.sbuf_tensor("cNL", [P, 1], _f32))   # -L
        rw = e(nc.sbuf_tensor("rw", [1, 16], _f32))
        acc2 = e(nc.psum_tensor("acc2", [1, 1], _f32))  # n1
        acc3 = e(nc.psum_tensor("acc3", [1, 1], _f32))  # s1
        accA = e(nc.psum_tensor("accA", [1, 1], _f32))  # A
        acc4 = e(nc.psum_tensor("acc4", [1, 1], _f32))  # s2
        accS = e(nc.psum_tensor("accS", [1, 1], _f32))  # S
        s_in = e(nc.semaphore("s_in"))
        s_p = e(nc.semaphore("s_p"))      # cast(+1) then sigmoid(+1)
        s_n1 = e(nc.semaphore("s_n1"))    # n1 col ready
        s_S = e(nc.semaphore("s_S"))      # S col ready (ACT Scopy)
        s_pe0 = e(nc.semaphore("s_pe0"))  # mm0 (n1 total) done
        s_pe1 = e(nc.semaphore("s_pe1"))  # mm1 (s1 total) done
        s_d = e(nc.semaphore("s_d"))      # ACT scalar helpers
        s_q = e(nc.semaphore("s_q"))      # nsq -> G0 RAW edge
        s_f = e(nc.semaphore("s_f"))      # k2 + kA (out gate)
        s_v = e(nc.semaphore("s_v"))      # global counting sem

        pred_v = xt[:, 0:F]
        true_v = xt[:, F : 2 * F].bitcast(_i32)

        n1_c = acc2[0:1, 0:1]
        s1_c = acc3[0:1, 0:1]
        A_c = accA[0:1, 0:1]
        s2_c = acc4[0:1, 0:1]
        S_c = accS[0:1, 0:1]

        def c(i):  # epilogue scratch cells in rw
            return rw[0:1, i : i + 1]

        d_cell = c(0)
        den_cell = c(1)
        nsq_cell = c(13)
        u_cell = c(2)       # L - 2*n1
        cn1_cell = c(3)     # n1
        cs1d_cell = c(4)    # 2*s1
        ncs1d_cell = c(5)   # -2*s1
        s1mL_cell = c(6)    # s1 - L
        r_cell = c(7)
        g0_cell = c(8)
        k1_cell = c(9)
        k2_cell = c(10)
        k3_cell = c(11)
        out_cell = c(12)
        one_c = ones[0:1, 0:1]

        # ---- SP: input DMA first, result store last -------------------
        nc.sync.dma_start(xt[:], xin_d[:]).then_inc(s_in, 16)
        reg = nc.sync.alloc_register()
        nc.sync.reg_load(reg, out_cell.bitcast(_i32))._wait_ge(s_v, 10)
        nc.sync.store(out_d[0:1, 0:1].bitcast(_i32), reg)

        # ---- ACT: sigmoid, S column, 0ns scalar helpers ---------------
        # bias 0.0 resolves to the kept const-0.0 tile.
        nc.scalar.activation(p[:], pred_v, _Act.Sigmoid)._wait_ge(
            s_in, 16
        ).then_inc(s_p, 1)
        # A' column: Square-with-accum of (p+1) -> sum (p+1)^2 = A + L
        # (own-engine RAW on p via s_p; bias tile is our ones memset)
        nc.scalar.activation(
            sc[:], p[:], _Act.Square, bias=ones[:, 0:1], scale=1.0,
            accum_out=stats[:, 3:4],
        )._wait_ge(s_p, 2).then_inc(s_S, 1)
        # d = L - n1 ; den = n1*d (n1 from mm0's PSUM)
        nc.scalar.activation(
            d_cell, n1_c, _Act.Identity, scale=-1.0, bias=cL[0:1, 0:1]
        )._wait_ge(s_pe0, 1).then_inc(s_d, 1)
        nc.scalar.activation(
            den_cell, n1_c, _Act.Copy, scale=d_cell
        )._wait_ge(s_d, 1).then_inc(s_d, 1)  # RAW on d
        # u = L - 2*n1 ; cn1 = n1 (in-order after d covers mm0)
        nc.scalar.activation(
            u_cell, n1_c, _Act.Identity, scale=-2.0, bias=cL[0:1, 0:1]
        ).then_inc(s_d, 1)
        nc.scalar.activation(cn1_cell, n1_c, _Act.Copy).then_inc(s_d, 1)
        # cs1d = 2*s1 ; negcs1d = -2*s1 ; s1mL = s1 - L (via mm1's PSUM)
        nc.scalar.activation(
            cs1d_cell, s1_c, _Act.Copy, scale=2.0
        )._wait_ge(s_pe1, 1).then_inc(s_d, 1)
        nc.scalar.activation(
            ncs1d_cell, s1_c, _Act.Copy, scale=-2.0
        ).then_inc(s_d, 1)
        nc.scalar.activation(
            s1mL_cell, s1_c, _Act.Identity, scale=1.0, bias=cNL[0:1, 0:1]
        ).then_inc(s_d, 1)  # s_d total = 7
        # kA = n1*A' + k1 on ACT (count 9 provably forces mmA and k1;
        # ACT's sem send is 2ns cheaper and this unblocks k2 on DVE)
        nc.scalar.activation(
            k3_cell, A_c, _Act.Identity, scale=cn1_cell, bias=k1_cell
        )._wait_ge(s_v, 9).then_inc(s_f, 1)

        # ---- DVE: consts, cast + n1 sums, products, epilogue ----------
        nc.vector.memset(ones[:], 1.0)
        nc.vector.memset(cL[:], float(L))
        nc.vector.memset(cNL[:], -float(L))
        nc.vector.tensor_copy(m1[:], true_v)._wait_ge(s_in, 16).then_inc(
            s_p, 1
        )  # int -> f32 cast (values 0/1)
        nc.vector.tensor_reduce(
            ncol[:], m1[:], axis=_X, op=_Alu.add
        )._wait_ge(s_p, 1).then_inc(s_n1, 1)  # covers the memsets too
        # mp = m*p, row sums -> s1 col (stats0)
        nc.vector.scalar_tensor_tensor(
            out=mp[:], in0=m1[:], scalar=1.0, in1=p[:],
            op0=_Alu.mult, op1=_Alu.mult, accum_out=stats[:, 0:1],
        )._wait_ge(s_p, 2).then_inc(s_v, 1)  # 1: needs cast AND sigmoid
        # pS: copy p with row-sum accum -> S col (stats1); tensor_scalar
        # runs in the DVE 2x_2p mode (93ns) and fills the mp->mpp RAW gap
        nc.vector.tensor_scalar(
            out=p2A[:], in0=p[:], scalar1=1.0, scalar2=0.0,
            op0=_Alu.mult, op1=_Alu.add, accum_out=stats[:, 1:2],
        ).then_inc(s_v, 1)  # 2
        # mpp = mp*p, row sums -> s2 col (stats2); RAW on mp via s_v
        nc.vector.scalar_tensor_tensor(
            out=mpp[:], in0=mp[:], scalar=1.0, in1=p[:],
            op0=_Alu.mult, op1=_Alu.mult, accum_out=stats[:, 2:3],
        )._wait_ge(s_v, 1).then_inc(s_v, 1)  # 3
        # nsq = n1*n1 (den - L*n1 == -n1^2 folds the A' +L shift); uses
        # the SBUF n1 bounce as the scalar so only one PSUM operand
        nc.vector.scalar_tensor_tensor(
            out=nsq_cell, in0=n1_c, scalar=cn1_cell, in1=one_c,
            op0=_Alu.mult, op1=_Alu.mult,
        )._wait_ge(s_d, 4).then_inc(s_q, 1)  # cn1 is the 4th helper
        # r = 1/den; the s_d>=7 wait orders ALL ACT helper cells for the
        # in-order epilogue ops below.
        nc.vector.reciprocal(r_cell, den_cell)._wait_ge(s_d, 7).then_inc(
            s_v, 1
        )  # 5
        # G0 = 2*s1*(s1-L) - n1^2
        nc.vector.scalar_tensor_tensor(
            out=g0_cell, in0=cs1d_cell, scalar=s1mL_cell, in1=nsq_cell,
            op0=_Alu.mult, op1=_Alu.subtract,
        )._wait_ge(s_q, 1).then_inc(s_v, 1)  # 5
        # k1 = -2*s1*S + G0   (count 7 <=> G0 AND mmS both retired)
        nc.vector.scalar_tensor_tensor(
            out=k1_cell, in0=S_c, scalar=ncs1d_cell, in1=g0_cell,
            op0=_Alu.mult, op1=_Alu.add,
        )._wait_ge(s_v, 6).then_inc(s_v, 1)
        # k2 = (L-2*n1)*s2    (count 8 forces mm2 via PE pipeline order)
        nc.vector.scalar_tensor_tensor(
            out=k2_cell, in0=s2_c, scalar=u_cell, in1=one_c,
            op0=_Alu.mult, op1=_Alu.mult,
        )._wait_ge(s_v, 8).then_inc(s_f, 1)
        # out = (k2 + kA) * r  ==  (num + den)/den; kA comes from ACT
        nc.vector.scalar_tensor_tensor(
            out=out_cell, in0=k2_cell, scalar=k3_cell, in1=r_cell,
            op0=_Alu.add, op1=_Alu.mult,
        )._wait_ge(s_f, 2).then_inc(s_v, 1)  # 10

        # ---- PE: five single-column partition reductions --------------
        nc.tensor.matmul(
            acc2[0:1, 0:1], ones[:], ncol[:], start=True, stop=True
        )._wait_ge(s_n1, 1).then_inc(s_pe0, 1)
        nc.tensor.matmul(
            acc3[0:1, 0:1], ones[:], stats[:, 0:1], start=True, stop=True
        )._wait_ge(s_v, 1).then_inc(s_pe1, 1)
        nc.tensor.matmul(
            accS[0:1, 0:1], ones[:], stats[:, 1:2], start=True, stop=True
        )._wait_ge(s_v, 2).then_inc(s_v, 1)  # S total (early)
        nc.tensor.matmul(
            acc4[0:1, 0:1], ones[:], stats[:, 2:3], start=True, stop=True
        )._wait_ge(s_v, 3).then_inc(s_v, 1)  # s2 total
        nc.tensor.matmul(
            accA[0:1, 0:1], ones[:], stats[:, 3:4], start=True, stop=True
        )._wait_ge(s_S, 1).then_inc(s_v, 1)  # A' total (last)

        # ---- Pool tail: reset DMA queues + semaphores -----------------
        # s_v>=12 implies every semaphore reached its final value (the SP
        # store itself carries no sem ops, so clearing concurrently with
        # it is safe for re-execution).
        sems = (s_in, s_p, s_n1, s_S, s_pe0, s_pe1, s_d, s_q, s_f, s_v)
        sem_lo = min(s.num for s in sems)
        sem_hi = max(s.num for s in sems)
        nc.gpsimd.dma_reset(range(sem_lo, sem_hi + 1))._wait_ge(s_v, 10)
        nc.gpsimd.sem_clear(range(sem_lo, sem_hi + 1))

    nc.compile()
    return nc


def _pack(pred_Y, true_Y):
    xin = np.empty((P, 2 * F), dtype=np.float32)
    xin[:, 0:F] = np.ascontiguousarray(pred_Y, dtype=np.float32).reshape(P, F)
    xin[:, F : 2 * F] = (
        np.ascontiguousarray(true_Y, dtype=np.int32).reshape(P, F).view(np.float32)
    )
    return xin


def _run(pred_Y, true_Y, **hw_kwargs):
    global _built
    if _built is None:
        _built = _build()
    in_map = {"xin": _pack(pred_Y, true_Y)}
    res = run_bass_kernel_spmd(
        _built, [in_map] * N_CORES, list(range(N_CORES)), **hw_kwargs
    )
    out = np.asarray(res.results[0]["out"], dtype=np.float32).reshape(())
    return out, res


def kernel(pred_Y, true_Y):
    out, _ = _run(pred_Y, true_Y)
    return out


# revision 9
# speedup vs baseline: 1.0003x; 1.0003x over previous
"""Trainium2 Bass kernel for nn_DIYloss_1709396984424 — v2.

Loss: for binary labels, mean over (one, zero) pairs of (1 + p[l] - p[k])^2
where p = sigmoid(pred_Y). The L^2 pairwise sum collapses to O(L) masked
reductions. With n1 = sum(m), s1 = sum(m*p), s2 = sum(m*p^2), S = sum(p),
A = sum(p^2 + 2p):

    num + den = n1*A + (L - 2*n1)*s2 + 2*s1^2 - 2*L*s1 - 2*s1*S + n1*(L - n1)
    loss      = (num + den) / den   where den = n1*(L - n1)

Schedule: five tiny PE partition-reductions fire the moment each stats
column lands, and the scalar epilogue is a running sum that picks up each
total in arrival order, so only ~50ns of scalar chain sits after the last
column total:
  SP  : input DMA first, final 4-byte reg store.
  ACT : act-table load (auto), sigmoid, Square-with-accum of (p+1) for
        the A' = A + L column, then eight ENGINE-0ns scalar ops gated by
        the early matmuls: d = L - n1, den = n1*d, u = L - 2*n1,
        cn1 = n1, cs1d = 2*s1, negcs1d = -2*s1, s1mL = s1 - L, and the
        late merge kA = n1*A' + k1 (ACT's sem send beats DVE's by 2ns
        and it unblocks k2 on DVE).
  DVE : const memsets, mask cast + n1 row sums (hidden in the DMA wait),
        mp = m*p (accum s1), pS = copy p (accum S; tensor_scalar runs in
        the 2x DVE mode at 93ns and fills the mp->mpp RAW gap),
        mpp = mp*p (accum s2), then the ENGINE-0ns epilogue
        nsq = n1^2 (den - L*n1 == -n1^2 absorbs the A' shift), r = 1/den,
        G0 = 2*s1*(s1-L) - nsq, k1 = -2*s1*S + G0, k2 = (L-2*n1)*s2,
        out = (k2 + kA)*r gated on a 2-count sem that provably forces
        both final merges.
  PE  : mm0 (n1, ~1.5us early), mm1 (s1), mmS (S), mm2 (s2), mmA (A',
        last) — each a [128,1]x[128,1] reduction into its own PSUM bank.
  Pool: nothing but the end-of-program DMA-queue + semaphore reset (its
        compute ops are not supported by this ISA).

ISA constraints learned the hard way: a DVE op may touch at most ONE PSUM
operand (two PSUM tensors fail BIR verification; PSUM scalars off
partition 0 too), and Pool/GPSIMD cannot run TensorScalarPtr at all. All
cross-total products therefore pair one PSUM total with SBUF cells that
ACT bounced out of earlier PSUM results (ACT scalar ops cost 0ns).

Hazard rules: engine pipelines do NOT interlock same-engine RAW on small
operands; every RAW dependency carries a semaphore edge (one wait per
instruction; counting sems + in-order retirement cover multi-producer
cases; cross-engine producers share a counting sem so one wait covers
both — a count of N implies all N increments happened, regardless of
arrival order).

The framework's startup barrier and three of its four const-tensor memsets
are suppressed at module-build time (the sigmoid bias uses the kept
const-0.0). The end-of-program reset rides on the final-value semaphore
rather than a store-completion sem so it overlaps the SP store.
"""

import numpy as np

try:
    import concourse.bass as cbass  # noqa: F401
except ImportError:  # pragma: no cover - grading env should have it on path
    import sys

    sys.path.insert(0, "/opt/trn_rl_repo")
    import concourse.bass as cbass  # noqa: F401

from concourse import bacc, mybir
from concourse.bass_utils import run_bass_kernel_spmd

L = 8192
P = 128
F = L // P  # 64
N_CORES = 8

_f32 = mybir.dt.float32
_i32 = mybir.dt.int32
_Alu = mybir.AluOpType
_Act = mybir.ActivationFunctionType
_X = mybir.AxisListType.X

_built = None

_suppress = {"on": False}
_orig_memset = cbass.BassSharedVectorInterface.memset
_orig_aeb = cbass.Bass.all_engine_barrier


def _memset_patched(self, ap, constant):
    if _suppress["on"] and constant != 0.0:
        return None
    return _orig_memset(self, ap, constant)


def _aeb_patched(self, *a, **k):
    if _suppress["on"]:
        return None
    return _orig_aeb(self, *a, **k)


cbass.BassSharedVectorInterface.memset = _memset_patched
cbass.Bass.all_engine_barrier = _aeb_patched


def _build():
    _suppress["on"] = True
    try:
        nc = bacc.Bacc(
            "TRN2", debug=False, target_bir_lowering=False, num_devices=N_CORES
        )
    finally:
        _suppress["on"] = False

    # cols 0:F = pred_Y (f32), cols F:2F = true_Y (int32 bitcast to f32)
    xin_d = nc.dram_tensor("xin", [P, 2 * F], _f32, kind="ExternalInput")
    out_d = nc.dram_tensor("out", [1, 1], _f32, kind="ExternalOutput")

    from contextlib import ExitStack

    with ExitStack() as stack:
        e = stack.enter_context
        xt = e(nc.sbuf_tensor("xt", [P, 2 * F], _f32))
        p = e(nc.sbuf_tensor("p", [P, F], _f32))
        m1 = e(nc.sbuf_tensor("m1", [P, F], _f32))
        mp = e(nc.sbuf_tensor("mp", [P, F], _f32))
        mpp = e(nc.sbuf_tensor("mpp", [P, F], _f32))
        p2A = e(nc.sbuf_tensor("p2A", [P, F], _f32))
        sc = e(nc.sbuf_tensor("sc", [P, F], _f32))  # Scopy dummy out
        stats = e(nc.sbuf_tensor("stats", [P, 4], _f32))
        ncol = e(nc.sbuf_tensor("ncol", [P, 1], _f32))
        ones = e(nc.sbuf_tensor("ones", [P, 1], _f32))
        cL = e(nc.sbuf_tensor("cL", [P, 1], _f32))     # +L
        cNL = e(nc.sbuf_tensor("cNL", [P, 1], _f32))   # -L
        rw = e(nc.sbuf_tensor("rw", [1, 16], _f32))
        acc2 = e(nc.psum_tensor("acc2", [1, 1], _f32))  # n1
        acc3 = e(nc.psum_tensor("acc3", [1, 1], _f32))  # s1
        accA = e(nc.psum_tensor("accA", [1, 1], _f32))  # A
        acc4 = e(nc.psum_tensor("acc4", [1, 1], _f32))  # s2
        accS = e(nc.psum_tensor("accS", [1, 1], _f32))  # S
        s_in = e(nc.semaphore("s_in"))
        s_p = e(nc.semaphore("s_p"))      # cast(+1) then sigmoid(+1)
        s_n1 = e(nc.semaphore("s_n1"))    # n1 col ready
        s_S = e(nc.semaphore("s_S"))      # S col ready (ACT Scopy)
        s_pe0 = e(nc.semaphore("s_pe0"))  # mm0 (n1 total) done
        s_pe1 = e(nc.semaphore("s_pe1"))  # mm1 (s1 total) done
        s_d = e(nc.semaphore("s_d"))      # ACT scalar helpers
        s_q = e(nc.semaphore("s_q"))      # nsq -> G0 RAW edge
        s_f = e(nc.semaphore("s_f"))      # k2 + kA (out gate)
        s_v = e(nc.semaphore("s_v"))      # global counting sem

        pred_v = xt[:, 0:F]
        true_v = xt[:, F : 2 * F].bitcast(_i32)

        n1_c = acc2[0:1, 0:1]
        s1_c = acc3[0:1, 0:1]
        A_c = accA[0:1, 0:1]
        s2_c = acc4[0:1, 0:1]
        S_c = accS[0:1, 0:1]

        def c(i):  # epilogue scratch cells in rw
            return rw[0:1, i : i + 1]

        d_cell = c(0)
        den_cell = c(1)
        nsq_cell = c(13)
        u_cell = c(2)       # L - 2*n1
        cn1_cell = c(3)     # n1
        cs1d_cell = c(4)    # 2*s1
        ncs1d_cell = c(5)   # -2*s1
        s1mL_cell = c(6)    # s1 - L
        r_cell = c(7)
        g0_cell = c(8)
        k1_cell = c(9)
        k2_cell = c(10)
        k3_cell = c(11)
        out_cell = c(12)
        one_c = ones[0:1, 0:1]

        # ---- SP: input DMA first, result store last -------------------
        nc.sync.dma_start(xt[:], xin_d[:]).then_inc(s_in, 16)
        reg = nc.sync.alloc_register()
        nc.sync.reg_load(reg, out_cell.bitcast(_i32))._wait_ge(s_v, 10)
        nc.sync.store(out_d[0:1, 0:1].bitcast(_i32), reg)

        # ---- ACT: sigmoid, S column, 0ns scalar helpers ---------------
        # bias 0.0 resolves to the kept const-0.0 tile.
        nc.scalar.activation(p[:], pred_v, _Act.Sigmoid)._wait_ge(
            s_in, 16
        ).then_inc(s_p, 1)
        # A' column: Square-with-accum of (p+1) -> sum (p+1)^2 = A + L
        # (own-engine RAW on p via s_p; bias tile is our ones memset)
        nc.scalar.activation(
            sc[:], p[:], _Act.Square, bias=ones[:, 0:1], scale=1.0,
            accum_out=stats[:, 3:4],
        )._wait_ge(s_p, 2).then_inc(s_S, 1)
        # d = L - n1 ; den = n1*d (n1 from mm0's PSUM)
        nc.scalar.activation(
            d_cell, n1_c, _Act.Identity, scale=-1.0, bias=cL[0:1, 0:1]
        )._wait_ge(s_pe0, 1).then_inc(s_d, 1)
        nc.scalar.activation(
            den_cell, n1_c, _Act.Copy, scale=d_cell
        )._wait_ge(s_d, 1).then_inc(s_d, 1)  # RAW on d
        # u = L - 2*n1 ; cn1 = n1 (in-order after d covers mm0)
        nc.scalar.activation(
            u_cell, n1_c, _Act.Identity, scale=-2.0, bias=cL[0:1, 0:1]
        ).then_inc(s_d, 1)
        nc.scalar.activation(cn1_cell, n1_c, _Act.Copy).then_inc(s_d, 1)
        # cs1d = 2*s1 ; negcs1d = -2*s1 ; s1mL = s1 - L (via mm1's PSUM)
        nc.scalar.activation(
            cs1d_cell, s1_c, _Act.Copy, scale=2.0
        )._wait_ge(s_pe1, 1).then_inc(s_d, 1)
        nc.scalar.activation(
            ncs1d_cell, s1_c, _Act.Copy, scale=-2.0
        ).then_inc(s_d, 1)
        nc.scalar.activation(
            s1mL_cell, s1_c, _Act.Identity, scale=1.0, bias=cNL[0:1, 0:1]
        ).then_inc(s_d, 1)  # s_d total = 7
        # kA = n1*A' + k1 on ACT (count 9 provably forces mmA and k1;
        # ACT's sem send is 2ns cheaper and this unblocks k2 on DVE)
        nc.scalar.activation(
            k3_cell, A_c, _Act.Identity, scale=cn1_cell, bias=k1_cell
        )._wait_ge(s_v, 9).then_inc(s_f, 1)
        # out = kA*r + k2r  ==  (num + den)/den; the s_f>=2 gate forces
        # both final merges (and carries kA's write-drain edge)
        nc.scalar.activation(
            out_cell, k3_cell, _Act.Identity, scale=r_cell, bias=k2_cell
        )._wait_ge(s_f, 2).then_inc(s_v, 1)  # 10

        # ---- DVE: consts, cast + n1 sums, products, epilogue ----------
        nc.vector.memset(ones[:], 1.0)
        nc.vector.memset(cL[:], float(L))
        nc.vector.memset(cNL[:], -float(L))
        nc.vector.tensor_copy(m1[:], true_v)._wait_ge(s_in, 16).then_inc(
            s_p, 1
        )  # int -> f32 cast (values 0/1)
        nc.vector.tensor_reduce(
            ncol[:], m1[:], axis=_X, op=_Alu.add
        )._wait_ge(s_p, 1).then_inc(s_n1, 1)  # covers the memsets too
        # mp = m*p, row sums -> s1 col (stats0)
        nc.vector.scalar_tensor_tensor(
            out=mp[:], in0=m1[:], scalar=1.0, in1=p[:],
            op0=_Alu.mult, op1=_Alu.mult, accum_out=stats[:, 0:1],
        )._wait_ge(s_p, 2).then_inc(s_v, 1)  # 1: needs cast AND sigmoid
        # pS: copy p with row-sum accum -> S col (stats1); tensor_scalar
        # runs in the DVE 2x_2p mode (93ns) and fills the mp->mpp RAW gap
        nc.vector.tensor_scalar(
            out=p2A[:], in0=p[:], scalar1=1.0, scalar2=0.0,
            op0=_Alu.mult, op1=_Alu.add, accum_out=stats[:, 1:2],
        ).then_inc(s_v, 1)  # 2
        # mpp = mp*p, row sums -> s2 col (stats2); RAW on mp via s_v
        nc.vector.scalar_tensor_tensor(
            out=mpp[:], in0=mp[:], scalar=1.0, in1=p[:],
            op0=_Alu.mult, op1=_Alu.mult, accum_out=stats[:, 2:3],
        )._wait_ge(s_v, 1).then_inc(s_v, 1)  # 3
        # nsq = n1*n1 (den - L*n1 == -n1^2 folds the A' +L shift); uses
        # the SBUF n1 bounce as the scalar so only one PSUM operand
        nc.vector.scalar_tensor_tensor(
            out=nsq_cell, in0=n1_c, scalar=cn1_cell, in1=one_c,
            op0=_Alu.mult, op1=_Alu.mult,
        )._wait_ge(s_d, 4).then_inc(s_q, 1)  # cn1 is the 4th helper
        # r = 1/den; the s_d>=7 wait orders ALL ACT helper cells for the
        # in-order epilogue ops below.
        nc.vector.reciprocal(r_cell, den_cell)._wait_ge(s_d, 7).then_inc(
            s_v, 1
        )  # 5
        # G0 = 2*s1*(s1-L) - n1^2
        nc.vector.scalar_tensor_tensor(
            out=g0_cell, in0=cs1d_cell, scalar=s1mL_cell, in1=nsq_cell,
            op0=_Alu.mult, op1=_Alu.subtract,
        )._wait_ge(s_q, 1).then_inc(s_v, 1)  # 5
        # k1 = -2*s1*S + G0   (count 7 <=> G0 AND mmS both retired)
        nc.vector.scalar_tensor_tensor(
            out=k1_cell, in0=S_c, scalar=ncs1d_cell, in1=g0_cell,
            op0=_Alu.mult, op1=_Alu.add,
        )._wait_ge(s_v, 6).then_inc(s_v, 1)
        # k2r = (L-2*n1)*s2*r  (count 8 forces mm2 via PE pipeline
        # order; the in1 slot carries r so the final merge needs no
        # further multiply)
        nc.vector.scalar_tensor_tensor(
            out=k2_cell, in0=s2_c, scalar=u_cell, in1=r_cell,
            op0=_Alu.mult, op1=_Alu.mult,
        )._wait_ge(s_v, 8).then_inc(s_f, 1)

        # ---- PE: five single-column partition reductions --------------
        nc.tensor.matmul(
            acc2[0:1, 0:1], ones[:], ncol[:], start=True, stop=True
        )._wait_ge(s_n1, 1).then_inc(s_pe0, 1)
        nc.tensor.matmul(
            acc3[0:1, 0:1], ones[:], stats[:, 0:1], start=True, stop=True
        )._wait_ge(s_v, 1).then_inc(s_pe1, 1)
        nc.tensor.matmul(
            accS[0:1, 0:1], ones[:], stats[:, 1:2], start=True, stop=True
        )._wait_ge(s_v, 2).then_inc(s_v, 1)  # S total (early)
        nc.tensor.matmul(
            acc4[0:1, 0:1], ones[:], stats[:, 2:3], start=True, stop=True
        )._wait_ge(s_v, 3).then_inc(s_v, 1)  # s2 total
        nc.tensor.matmul(
            accA[0:1, 0:1], ones[:], stats[:, 3:4], start=True, stop=True
        )._wait_ge(s_S, 1).then_inc(s_v, 1)  # A' total (last)

        # ---- Pool tail: reset DMA queues + semaphores -----------------
        # s_v>=12 implies every semaphore reached its final value (the SP
        # store itself carries no sem ops, so clearing concurrently with
        # it is safe for re-execution).
        sems = (s_in, s_p, s_n1, s_S, s_pe0, s_pe1, s_d, s_q, s_f, s_v)
        sem_lo = min(s.num for s in sems)
        sem_hi = max(s.num for s in sems)
        nc.gpsimd.dma_reset(range(sem_lo, sem_hi + 1))._wait_ge(s_v, 10)
        nc.gpsimd.sem_clear(range(sem_lo, sem_hi + 1))

    nc.compile()
    return nc


def _pack(pred_Y, true_Y):
    xin = np.empty((P, 2 * F), dtype=np.float32)
    xin[:, 0:F] = np.ascontiguousarray(pred_Y, dtype=np.float32).reshape(P, F)
    xin[:, F : 2 * F] = (
        np.ascontiguousarray(true_Y, dtype=np.int32).reshape(P, F).view(np.float32)
    )
    return xin


def _run(pred_Y, true_Y, **hw_kwargs):
    global _built
    if _built is None:
        _built = _build()
    in_map = {"xin": _pack(pred_Y, true_Y)}
    res = run_bass_kernel_spmd(
        _built, [in_map] * N_CORES, list(range(N_CORES)), **hw_kwargs
    )
    out = np.asarray(res.results[0]["out"], dtype=np.float32).reshape(())
    return out, res


def kernel(pred_Y, true_Y):
    out, _ = _run(pred_Y, true_Y)
    return out
